# revision 29
# baseline (speedup 1.0000x reference)
"""Trainium2 Bass kernel for causal self-attention (dense transformer block attn).

Reference computation (per batch b):
    qkv = x @ W_attn + b_attn ; split into per-head Q, K, V (16 heads, hs=64)
    att = softmax(mask(Q K^T / sqrt(hs))) ; y = att @ V ; out = y @ W_proj + b_proj

Sharding (8 cores): data parallel on B (2) x tensor parallel on head groups
(4 groups of 4 heads, Megatron-style column/row split of W_attn / W_proj).
Each core computes a partial out^T [1024, 2048] (f32); host sums the 4 partials
per batch, adds b_proj (+ the folded V-bias term b_v @ W_proj) and transposes.

Layout / algebra notes:
  - K bias is dropped entirely (per-q-constant shift, softmax-invariant);
    V bias is dropped on-device and folded into the host epilogue as
    b_v @ W_proj.  Only the Q bias is applied on-chip.
  - q-block-major schedule: x arrives per 512-wide q block, and the whole
    pipeline (qkv -> scores -> exp -> PV -> proj -> out DMA) streams per
    block, so the PE starts ~1us in and never waits for the full input.
  - V is produced directly in natural [kpos, hs] layout by swapping the
    matmul operands (stationary = x^T chunk, moving = W_v columns): no PE
    transposes.
  - scores are computed as S^T = K Q^T with k-positions on partitions; the
    two heads of a pair run as row-tiled (tile_position) K=64 matmuls that
    execute concurrently on the PE sub-arrays.
  - softmax denominator: PV stationary is [V | ones] ([ones | V] for the
    other head) so the other 64 psum partitions accumulate copies of
    sum_k P; normalization is reciprocal_approx_fast (single DVE op,
    ~18 bits) + one multiply -- no DMA bounce.
"""

import numpy as np
import ml_dtypes

import concourse.bass as bass
import concourse.tile as tile
import concourse.mybir as mybir
from concourse import bacc
from concourse.bass_utils import run_bass_kernel_spmd

BF16 = mybir.dt.bfloat16
F32 = mybir.dt.float32
AF = mybir.ActivationFunctionType

T = 2048          # sequence length
C = 1024          # model dim
HPC = 4           # heads per core
HS = 64           # head size
NF = 3 * HPC * HS  # per-core qkv features (768)
N_CORES = 8
QB = 512          # q block (psum bank of f32)
LAG = 2           # slots the PV matmuls trail the exp that feeds them
DEBUG = False

bf16 = ml_dtypes.bfloat16


def build_kernel():
    nc = bacc.Bacc("TRN2", target_bir_lowering=False, debug=False)

    xT = nc.dram_tensor("xT", [C, T], BF16, kind="ExternalInput").ap()
    W = nc.dram_tensor("W", [C, NF], BF16, kind="ExternalInput").ap()
    bq = nc.dram_tensor("bq", [128, 2], F32, kind="ExternalInput").ap()
    Wp = nc.dram_tensor("Wp", [HPC * HS, C], BF16, kind="ExternalInput").ap()
    mask = nc.dram_tensor("mask", [128, 128], BF16, kind="ExternalInput").ap()
    outT = nc.dram_tensor("outT", [C, T], BF16, kind="ExternalOutput").ap()
    dbg = {
        "dbg_q": nc.dram_tensor("dbg_q", [128, 2, T], BF16,
                                kind="ExternalOutput").ap(),
        "dbg_k": nc.dram_tensor("dbg_k", [128, 2, T], BF16,
                                kind="ExternalOutput").ap(),
        "dbg_y": nc.dram_tensor("dbg_y", [128, 2, T], BF16,
                                kind="ExternalOutput").ap(),
        "dbg_v": nc.dram_tensor("dbg_v", [128, 2 * 16 * 2 * 128], BF16,
                                kind="ExternalOutput").ap(),
    } if DEBUG else None

    with tile.TileContext(nc) as tc:
        _emit(nc, tc, xT, W, bq, Wp, mask, outT, dbg)
    nc.compile()
    return nc


def _emit(nc, tc, xT, W, bq, Wp, mask, outT, dbg=None):
    from contextlib import ExitStack

    ctx = ExitStack()
    consts = ctx.enter_context(tc.tile_pool(name="consts", bufs=1))
    pt_pool = ctx.enter_context(tc.tile_pool(name="pt", bufs=4))
    rt_pool = ctx.enter_context(tc.tile_pool(name="rt", bufs=2))
    osb_pool = ctx.enter_context(tc.tile_pool(name="osb", bufs=4))
    ps_s = ctx.enter_context(tc.tile_pool(name="ps_s", bufs=2, space="PSUM"))
    ps_pv = ctx.enter_context(tc.tile_pool(name="ps_pv", bufs=2, space="PSUM"))
    ps_mm = ctx.enter_context(tc.tile_pool(name="ps_mm", bufs=2, space="PSUM"))

    # ---------------- input DMA (q-block-major streaming) ----------------
    xT_v = xT.rearrange("(c p) t -> p c t", p=128)
    W_v = W.rearrange("(c p) n -> p c n", p=128)
    W_t = consts.tile([128, 8, NF], BF16, tag="W", name="W_t")
    xq_t = consts.tile([128, 8, 4, QB], BF16, tag="xq", name="xq_t")
    # Q bias first (tiny, needed by the first Q evacuation), then W chunks
    # interleaved with the first q block's x chunks so the lead matmuls can
    # start as soon as (W_c, x_c0) pairs land.
    b_t = consts.tile([128, 2], F32, tag="b", name="b_t")
    nc.sync.dma_start(out=b_t, in_=bq)
    for c in range(8):
        nc.sync.dma_start(out=W_t[:, c, :], in_=W_v[:, c, :])
        nc.sync.dma_start(out=xq_t[:, c, 0, :], in_=xT_v[:, c, 0:QB])
    mask_t = consts.tile([128, 128], BF16, tag="mask", name="mask_t")
    nc.sync.dma_start(out=mask_t, in_=mask)
    # x blocks 1..3 as per-chunk DMAs behind the W/x0 stream on the same
    # queue: issue order gives the lead stream bandwidth priority, and
    # chunk granularity spreads each block across many DMA engines (a
    # single big DMA runs on one engine at a fraction of HBM bandwidth).
    for qb in range(1, 4):
        for c in range(8):
            nc.sync.dma_start(
                out=xq_t[:, c, qb, :], in_=xT_v[:, c, qb * QB:(qb + 1) * QB]
            )
    Wp_t = consts.tile([128, 2, C], BF16, tag="Wp", name="Wp_t")
    nc.sync.dma_start(out=Wp_t, in_=Wp.rearrange("(k p) n -> p k n", p=128))

    qT = consts.tile([128, 2, T], BF16, tag="qT", name="qT")
    kT = consts.tile([128, 2, T], BF16, tag="kT", name="kT")
    # vnat[p, pair, j, hl, col]: PV stationary tiles. hl=0: [V | ones],
    # hl=1: [ones | V] so that y lands on the partitions matching yT layout.
    vnat = consts.tile([128, 2, 16, 2, 128], BF16, tag="vnat", name="vnat")
    yT = consts.tile([128, 2, T], BF16, tag="yT", name="yT")

    # warm up the ACT exp table early so the ~2.7us load overlaps the lead-in
    warm = consts.tile([128, 8], F32, tag="warm", name="warm")
    nc.vector.memset(warm, 0.0)
    nc.scalar.activation(warm, warm, AF.Exp, scale=1.0)

    nc.vector.memset(vnat[:, :, :, 0, 64:128], 1.0)
    nc.vector.memset(vnat[:, :, :, 1, 0:64], 1.0)

    # warm up the PE HAM clock gate during the input-DMA wait: ~3.4us of
    # sustained matmul activity on zeroed scratch flips the clock from
    # 1.2GHz to 2.4GHz before the first real (DMA-paced, PE-bound-if-cold)
    # lead matmuls arrive.
    jw = consts.tile([128, 128], BF16, tag="jw", name="jw")
    jx = consts.tile([128, QB], BF16, tag="jx", name="jx")
    nc.vector.memset(jw, 0.0)
    nc.vector.memset(jx, 0.0)
    ps_junk0 = ps_mm.tile([128, QB], F32, tag="mm", name="ps_junk0")
    for _ in range(8):
        nc.tensor.matmul(ps_junk0, lhsT=jw, rhs=jx, start=True, stop=True)

    # ---------------- work-item helpers ----------------
    def qkv_group(nf, qb):
        # Q or K chunk nf for q/t block qb: (x[qb] @ W[:, cols])^T
        ps = ps_mm.tile([128, QB], F32, tag="mm", name="ps_qkv")
        for c in range(8):
            nc.tensor.matmul(
                ps,
                lhsT=W_t[:, c, nf * 128:(nf + 1) * 128],
                rhs=xq_t[:, c, qb, :],
                start=(c == 0),
                stop=(c == 7),
            )
        qsl = slice(qb * QB, (qb + 1) * QB)
        if nf < 2:
            nc.vector.tensor_scalar_add(qT[:, nf, qsl], ps, b_t[:, nf:nf + 1])
        else:
            nc.vector.tensor_copy(kT[:, nf - 2, qsl], ps)

    def vn_mm(ps, j, c):
        qb, r = j // 4, j % 4
        nc.tensor.matmul(
            ps,
            lhsT=xq_t[:, c, qb, 128 * r:128 * r + 128],
            rhs=W_t[:, c, 512:768],
            start=(c == 0),
            stop=(c == 7),
        )

    def vn_evac(ps, j):
        for p in range(2):
            # psum cols [128p:128p+64 | +64:+128] -> vnat[j, 0, 0:64] and
            # [j, 1, 64:128] in one strided copy (also f32 -> bf16)
            v0 = vnat[:, p, j, 0, 0:64]
            dst = bass.AP(tensor=v0.tensor, offset=v0.offset,
                          ap=[v0.ap[0], [192, 2], [1, 64]])
            s0 = ps[:, 128 * p:128 * p + 64]
            src = bass.AP(tensor=s0.tensor, offset=s0.offset,
                          ap=[s0.ap[0], [64, 2], [1, 64]])
            nc.vector.tensor_copy(dst, src)

    def vn_chunk(j):
        # V chunk j (128 k-positions) in natural [kpos, vfeat] layout:
        # stationary = x^T slice, moving = W_v columns.  Covers both pairs.
        ps = ps_mm.tile([128, QB], F32, tag="mm", name="ps_vn")
        for c in range(8):
            vn_mm(ps[:, 0:256], j, c)
        vn_evac(ps, j)

    mb = bass.AP(tensor=mask_t.tensor, offset=mask_t.offset,
                 ap=[mask_t.ap[0], [0, 2], [1, 128]])

    def sc(p, j, qb):
        # scores^T + exp for pair p, key chunk j, q block qb (both heads).
        # Returns (pt, q0): pt[:, hl, q0:512] holds exp'd scores.
        r = j - 4 * qb
        q0 = 128 * r if r >= 0 else 0
        ps = ps_s.tile([128, 2, QB], F32, tag="s", name="ps_s_t")
        for hl in range(2):
            nc.tensor.matmul(
                ps[:, hl, q0:QB],
                lhsT=kT[64 * hl:64 * hl + 64, p, j * 128:(j + 1) * 128],
                rhs=qT[64 * hl:64 * hl + 64, p, qb * QB + q0:(qb + 1) * QB],
                start=True,
                stop=True,
            )
        pt = pt_pool.tile([128, 2, QB], BF16, tag="pt", name=f"pt_{p}_{j}_{qb}")
        nc.scalar.activation(pt[:, :, q0:QB], ps[:, :, q0:QB], AF.Exp,
                             scale=0.125)
        if r >= 0:
            # zero the q < k triangle of the diagonal 128x128 (both heads via
            # a broadcast AP over the head dim)
            nc.gpsimd.tensor_mul(pt[:, :, q0:q0 + 128], pt[:, :, q0:q0 + 128],
                                 mb)
        return pt, q0

    def pv_mm(pv_ps, p, hl, j, pt, q0, first, last):
        nc.tensor.matmul(
            pv_ps[:, q0:QB],
            lhsT=vnat[:, p, j, hl, :],
            rhs=pt[:, hl, q0:QB],
            start=first,
            stop=last,
        )

    def norm(pv_ps, p, hl, qb):
        # yT = y / den via reciprocal_approx_fast.  The recip runs over all
        # 128 partitions (cost scales with free size only); the 64 rows that
        # hold y instead of den produce garbage that is never read.
        ysl = slice(64 * hl, 64 * hl + 64)
        dsl = slice(64 - 64 * hl, 128 - 64 * hl)
        rt = rt_pool.tile([128, QB], F32, tag="rt", name="rt")
        nc.vector.reciprocal_approx_fast(out=rt, in_=pv_ps)
        nc.vector.tensor_mul(
            yT[ysl, p, qb * QB:(qb + 1) * QB], pv_ps[ysl, :], rt[dsl, :]
        )

    outT_v = outT.rearrange("(n p) t -> p n t", p=128)

    ob_tiles = {}

    def prj(qb, i):
        # final projection rows [128i:128(i+1)] for q block qb
        qsl = slice(qb * QB, (qb + 1) * QB)
        ps = ps_mm.tile([128, QB], F32, tag="mm", name="ps_o")
        for kc in range(2):
            nc.tensor.matmul(
                ps,
                lhsT=Wp_t[:, kc, i * 128:(i + 1) * 128],
                rhs=yT[:, kc, qsl],
                start=(kc == 0),
                stop=(kc == 1),
            )
        ob = osb_pool.tile([128, QB], BF16, tag="ob", name="ob")
        # alternate the evacuation between DVE and the mostly-idle ScalarE
        if i % 2 == 0:
            nc.vector.tensor_copy(ob, ps)
        else:
            nc.scalar.copy(ob, ps)
        nc.sync.dma_start(out=outT_v[:, i, qsl], in_=ob)

    # ---------------- emission schedule ----------------
    # Units run in order (p=0,qb=0), (1,0), (0,1), (1,1), ... Each unit is a
    # j-sweep of scores+exp with PV lagged LAG slots behind; filler work
    # (next step's qkv/V prep, previous step's projection) is sprinkled into
    # the sweep to keep the PE dense while ACT works through the exps.
    def unit(p, qb, fillers, carry=None):
        # Returns a `finish` closure emitting this unit's PV tail + norms;
        # the caller passes it into the NEXT unit (as `carry`), which emits
        # it right after its first score, so the next unit's scores reach
        # the ACT queue without waiting behind this unit's PV tail.
        js = list(range(4 * qb + 4))
        nslots = len(js)
        lag = 1 if nslots <= 4 else LAG
        nf = len(fillers)
        pts = {}
        pv0 = ps_pv.tile([128, QB], F32, tag="pv", name=f"pv_{p}_{qb}_0")
        pv1 = ps_pv.tile([128, QB], F32, tag="pv", name=f"pv_{p}_{qb}_1")
        pvt = (pv0, pv1)
        done = 0
        for j in js:
            pts[j] = sc(p, j, qb)
            if j == 0 and carry is not None:
                carry()
            want = nf * (j + 1) // nslots
            while done < want:
                fillers[done]()
                done += 1
            if j - lag >= 0:
                jp = j - lag
                pt, q0 = pts.pop(jp)
                for hl in range(2):
                    pv_mm(pvt[hl], p, hl, jp, pt, q0, jp == 0,
                          jp == js[-1])

        def finish():
            # tail: interleave each hl's last PV with the other's norm so
            # the DVE normalize overlaps the remaining PE work
            for jp in range(max(0, len(js) - lag), len(js)):
                pt, q0 = pts.pop(jp)
                last = jp == js[-1]
                pv_mm(pvt[0], p, 0, jp, pt, q0, jp == 0, last)
                if last:
                    norm(pvt[0], p, 0, qb)
                pv_mm(pvt[1], p, 1, jp, pt, q0, jp == 0, last)
                if last:
                    norm(pvt[1], p, 1, qb)
        return finish

    with nc.named_scope("lead"):
        # All x0-dependent matmul work, interleaved per x chunk as the DMAs
        # land: Q/K for both pairs of q block 0 plus V chunks 0..3 -- eight
        # concurrent psum groups (2 ps_mm + 2 ps_pv + 2x2 ps_s bank halves)
        # so the PE is saturated from the first chunk's arrival.
        ps_k0 = ps_mm.tile([128, QB], F32, tag="mm", name="ps_qkv")
        ps_k1 = ps_mm.tile([128, QB], F32, tag="mm", name="ps_qkv")
        ps_q0 = ps_pv.tile([128, QB], F32, tag="pv", name="ps_q0")
        ps_q1 = ps_pv.tile([128, QB], F32, tag="pv", name="ps_q1")
        ps_v01 = ps_s.tile([128, 2, QB], F32, tag="s", name="ps_v01")
        ps_v23 = ps_s.tile([128, 2, QB], F32, tag="s", name="ps_v23")
        vps = [ps_v01[:, 0, 0:256], ps_v01[:, 1, 0:256],
               ps_v23[:, 0, 0:256], ps_v23[:, 1, 0:256]]
        for c in range(8):
            for ps, nf in ((ps_k0, 2), (ps_k1, 3), (ps_q0, 0), (ps_q1, 1)):
                nc.tensor.matmul(
                    ps,
                    lhsT=W_t[:, c, nf * 128:(nf + 1) * 128],
                    rhs=xq_t[:, c, 0, :],
                    start=(c == 0),
                    stop=(c == 7),
                )
            for j in range(4):
                vn_mm(vps[j], j, c)
        nc.scalar.copy(kT[:, 0, 0:QB], ps_k0)
        nc.scalar.copy(kT[:, 1, 0:QB], ps_k1)
        nc.scalar.add(qT[:, 0, 0:QB], ps_q0, b_t[:, 0:1])
        nc.scalar.add(qT[:, 1, 0:QB], ps_q1, b_t[:, 1:2])
        vn_evac(ps_v01[:, 0, :], 0)
        vn_evac(ps_v01[:, 1, :], 1)
        vn_evac(ps_v23[:, 0, :], 2)
        vn_evac(ps_v23[:, 1, :], 3)
        # q block 1's Q/K (x1-gated) ahead of the first unit
        qkv_group(0, 1)
        qkv_group(2, 1)

    def F(fn, *a):
        return lambda: fn(*a)

    # Units in q-block order 1, 2, 3, 0: ascending keeps the x DMA stream
    # ahead of compute, and saving block 0 for last makes the terminal
    # (ACT-bound) unit the smallest one.  Projections fill the ACT-heavy
    # late units.
    plan = [
        (0, 1, [F(vn_chunk, 4), F(vn_chunk, 5), F(qkv_group, 1, 1),
                F(vn_chunk, 6), F(vn_chunk, 7), F(qkv_group, 3, 1)]),
        (1, 1, [F(qkv_group, 0, 2), F(qkv_group, 2, 2)]
            + [F(vn_chunk, j) for j in range(8, 12)]),
        (0, 2, [F(qkv_group, 1, 2), F(qkv_group, 3, 2)]),
        (1, 2, [F(qkv_group, 0, 3), F(qkv_group, 2, 3)]),
        (0, 3, [F(qkv_group, 1, 3), F(qkv_group, 3, 3), F(vn_chunk, 12),
                F(vn_chunk, 13), F(vn_chunk, 14), F(vn_chunk, 15)]
            + [F(prj, 1, i) for i in range(6)]),
        (1, 3, [F(prj, 1, 6), F(prj, 1, 7)]
            + [F(prj, 2, i) for i in range(8)]),
        (0, 0, [F(prj, 3, i) for i in range(4)]),
        (1, 0, [F(prj, 3, i) for i in range(4, 8)]),
    ]
    carry = None
    for p, qb, fillers in plan:
        with nc.named_scope(f"u{p}_{qb}"):
            carry = unit(p, qb, fillers, carry)
    with nc.named_scope("tail"):
        carry()
        junk = ps_pv.tile([128, QB], F32, tag="pv", name="junk")
        for i in range(8):
            if i % 2 == 0:
                nc.tensor.matmul(junk, lhsT=W_t[:, 0, 0:128],
                                 rhs=xq_t[:, 0, 0, :], start=True,
                                 stop=True)
            prj(0, i)
    if dbg is not None:
        nc.sync.dma_start(out=dbg["dbg_q"], in_=qT)
        nc.sync.dma_start(out=dbg["dbg_k"], in_=kT)
        nc.sync.dma_start(out=dbg["dbg_y"], in_=yT)
        nc.sync.dma_start(out=dbg["dbg_v"],
                          in_=vnat.rearrange("p a b c d -> p (a b c d)"))
    ctx.close()


# ---------------------------------------------------------------------------
# host-side wrapper
# ---------------------------------------------------------------------------

_NC_CACHE = {}


def _get_nc():
    if "nc" not in _NC_CACHE:
        _NC_CACHE["nc"] = build_kernel()
    return _NC_CACHE["nc"]


def make_in_maps(x, W_attn, b_attn, W_proj, b_proj):
    # multiplicative causal mask for the diagonal chunk, [k, q]: 1 where q >= k
    mask_np = np.triu(np.ones((128, 128), np.float32)).astype(bf16)
    in_maps = []
    for core in range(N_CORES):
        b = core // 4
        g = core % 4
        cols = np.r_[256 * g:256 * g + 256,
                     1024 + 256 * g:1024 + 256 * g + 256,
                     2048 + 256 * g:2048 + 256 * g + 256]
        in_maps.append({
            "xT": np.ascontiguousarray(x[b].T).astype(bf16),
            "W": np.ascontiguousarray(W_attn[:, cols]).astype(bf16),
            "bq": np.ascontiguousarray(
                b_attn[256 * g:256 * g + 256].reshape(2, 128).T
            ).astype(np.float32),
            "Wp": np.ascontiguousarray(
                W_proj[256 * g:256 * g + 256, :]).astype(bf16),
            "mask": mask_np,
        })
    return in_maps


def kernel(x, W_attn, b_attn, W_proj, b_proj, _trace=False, _trace_kwargs=None):
    x = np.asarray(x, np.float32)
    W_attn = np.asarray(W_attn, np.float32)
    b_attn = np.asarray(b_attn, np.float32)
    W_proj = np.asarray(W_proj, np.float32)
    b_proj = np.asarray(b_proj, np.float32)

    nc = _get_nc()
    in_maps = make_in_maps(x, W_attn, b_attn, W_proj, b_proj)
    res = run_bass_kernel_spmd(
        nc, in_maps, core_ids=list(range(N_CORES)), trace=_trace,
        **(_trace_kwargs or {}),
    )
    B = x.shape[0]
    out = np.zeros((B, T, C), np.float32)
    for core in range(N_CORES):
        b = core // 4
        out[b] += res.results[core]["outT"].T.astype(np.float32)
    # K bias is softmax-invariant (dropped); V bias folds into the epilogue.
    out += (b_proj + b_attn[2 * C:] @ W_proj)[None, None, :]
    if _trace:
        kernel._last_results = res
    return out


if __name__ == "__main__":
    # smoke test: build only
    nc = build_kernel()
    print("built ok")


# revision 30
# speedup vs baseline: 1.0126x; 1.0126x over previous
"""Trainium2 Bass kernel for causal self-attention (dense transformer block attn).

Reference computation (per batch b):
    qkv = x @ W_attn + b_attn ; split into per-head Q, K, V (16 heads, hs=64)
    att = softmax(mask(Q K^T / sqrt(hs))) ; y = att @ V ; out = y @ W_proj + b_proj

Sharding (8 cores): data parallel on B (2) x tensor parallel on head groups
(4 groups of 4 heads, Megatron-style column/row split of W_attn / W_proj).
Each core computes a partial out^T [1024, 2048] (f32); host sums the 4 partials
per batch, adds b_proj (+ the folded V-bias term b_v @ W_proj) and transposes.

Layout / algebra notes:
  - K bias is dropped entirely (per-q-constant shift, softmax-invariant);
    V bias is dropped on-device and folded into the host epilogue as
    b_v @ W_proj.  Only the Q bias is applied on-chip.
  - q-block-major schedule: x arrives per 512-wide q block, and the whole
    pipeline (qkv -> scores -> exp -> PV -> proj -> out DMA) streams per
    block, so the PE starts ~1us in and never waits for the full input.
  - V is produced directly in natural [kpos, hs] layout by swapping the
    matmul operands (stationary = x^T chunk, moving = W_v columns): no PE
    transposes.
  - scores are computed as S^T = K Q^T with k-positions on partitions; the
    two heads of a pair run as row-tiled (tile_position) K=64 matmuls that
    execute concurrently on the PE sub-arrays.
  - softmax denominator: PV stationary is [V | ones] ([ones | V] for the
    other head) so the other 64 psum partitions accumulate copies of
    sum_k P; normalization is reciprocal_approx_fast (single DVE op,
    ~18 bits) + one multiply -- no DMA bounce.
"""

import numpy as np
import ml_dtypes

import concourse.bass as bass
import concourse.tile as tile
import concourse.mybir as mybir
from concourse import bacc
from concourse.bass_utils import run_bass_kernel_spmd

BF16 = mybir.dt.bfloat16
F32 = mybir.dt.float32
AF = mybir.ActivationFunctionType

T = 2048          # sequence length
C = 1024          # model dim
HPC = 4           # heads per core
HS = 64           # head size
NF = 3 * HPC * HS  # per-core qkv features (768)
N_CORES = 8
QB = 512          # q block (psum bank of f32)
LAG = 2           # slots the PV matmuls trail the exp that feeds them
DEBUG = False

bf16 = ml_dtypes.bfloat16


def build_kernel():
    nc = bacc.Bacc("TRN2", target_bir_lowering=False, debug=False)

    xT = nc.dram_tensor("xT", [C, T], BF16, kind="ExternalInput").ap()
    W = nc.dram_tensor("W", [C, NF], BF16, kind="ExternalInput").ap()
    bq = nc.dram_tensor("bq", [128, 2], F32, kind="ExternalInput").ap()
    Wp = nc.dram_tensor("Wp", [HPC * HS, C], BF16, kind="ExternalInput").ap()
    mask = nc.dram_tensor("mask", [128, 128], BF16, kind="ExternalInput").ap()
    outT = nc.dram_tensor("outT", [C, T], BF16, kind="ExternalOutput").ap()
    dbg = {
        "dbg_q": nc.dram_tensor("dbg_q", [128, 2, T], BF16,
                                kind="ExternalOutput").ap(),
        "dbg_k": nc.dram_tensor("dbg_k", [128, 2, T], BF16,
                                kind="ExternalOutput").ap(),
        "dbg_y": nc.dram_tensor("dbg_y", [128, 2, T], BF16,
                                kind="ExternalOutput").ap(),
        "dbg_v": nc.dram_tensor("dbg_v", [128, 2 * 16 * 2 * 128], BF16,
                                kind="ExternalOutput").ap(),
    } if DEBUG else None

    with tile.TileContext(nc) as tc:
        _emit(nc, tc, xT, W, bq, Wp, mask, outT, dbg)
    nc.compile()
    return nc


def _emit(nc, tc, xT, W, bq, Wp, mask, outT, dbg=None):
    from contextlib import ExitStack

    ctx = ExitStack()
    consts = ctx.enter_context(tc.tile_pool(name="consts", bufs=1))
    pt_pool = ctx.enter_context(tc.tile_pool(name="pt", bufs=4))
    rt_pool = ctx.enter_context(tc.tile_pool(name="rt", bufs=2))
    osb_pool = ctx.enter_context(tc.tile_pool(name="osb", bufs=4))
    ps_s = ctx.enter_context(tc.tile_pool(name="ps_s", bufs=2, space="PSUM"))
    ps_pv = ctx.enter_context(tc.tile_pool(name="ps_pv", bufs=2, space="PSUM"))
    ps_mm = ctx.enter_context(tc.tile_pool(name="ps_mm", bufs=2, space="PSUM"))

    # ---------------- input DMA (q-block-major streaming) ----------------
    xT_v = xT.rearrange("(c p) t -> p c t", p=128)
    W_v = W.rearrange("(c p) n -> p c n", p=128)
    W_t = consts.tile([128, 8, NF], BF16, tag="W", name="W_t")
    xq_t = consts.tile([128, 8, 4, QB], BF16, tag="xq", name="xq_t")
    # Q bias first (tiny, needed by the first Q evacuation), then W chunks
    # interleaved with the first q block's x chunks so the lead matmuls can
    # start as soon as (W_c, x_c0) pairs land.
    b_t = consts.tile([128, 2], F32, tag="b", name="b_t")
    nc.sync.dma_start(out=b_t, in_=bq)
    for c in range(8):
        nc.sync.dma_start(out=W_t[:, c, :], in_=W_v[:, c, :])
        nc.sync.dma_start(out=xq_t[:, c, 0, :], in_=xT_v[:, c, 0:QB])
    mask_t = consts.tile([128, 128], BF16, tag="mask", name="mask_t")
    nc.sync.dma_start(out=mask_t, in_=mask)
    # x blocks 1..3 as per-chunk DMAs behind the W/x0 stream on the same
    # queue: issue order gives the lead stream bandwidth priority, and
    # chunk granularity spreads each block across many DMA engines (a
    # single big DMA runs on one engine at a fraction of HBM bandwidth).
    for qb in range(1, 4):
        for c in range(8):
            nc.sync.dma_start(
                out=xq_t[:, c, qb, :], in_=xT_v[:, c, qb * QB:(qb + 1) * QB]
            )
    Wp_t = consts.tile([128, 2, C], BF16, tag="Wp", name="Wp_t")
    nc.sync.dma_start(out=Wp_t, in_=Wp.rearrange("(k p) n -> p k n", p=128))

    qT = consts.tile([128, 2, T], BF16, tag="qT", name="qT")
    kT = consts.tile([128, 2, T], BF16, tag="kT", name="kT")
    # vnat[p, pair, j, hl, col]: PV stationary tiles. hl=0: [V | ones],
    # hl=1: [ones | V] so that y lands on the partitions matching yT layout.
    vnat = consts.tile([128, 2, 16, 2, 128], BF16, tag="vnat", name="vnat")
    yT = consts.tile([128, 2, T], BF16, tag="yT", name="yT")

    # scratch for the HAM clock-gate pre-warm (memset first on the DVE
    # queue so the junk matmuls can start during the input-DMA wait)
    jw = consts.tile([128, 128], BF16, tag="jw", name="jw")
    jx = consts.tile([128, QB], BF16, tag="jx", name="jx")
    nc.vector.memset(jw, 0.0)
    nc.vector.memset(jx, 0.0)

    # warm up the ACT exp table early so the ~2.7us load overlaps the lead-in
    warm = consts.tile([128, 8], F32, tag="warm", name="warm")
    nc.vector.memset(warm, 0.0)
    nc.scalar.activation(warm, warm, AF.Exp, scale=1.0)

    nc.vector.memset(vnat[:, :, :, 0, 64:128], 1.0)
    nc.vector.memset(vnat[:, :, :, 1, 0:64], 1.0)

    # ---------------- work-item helpers ----------------
    def qkv_group(nf, qb):
        # Q or K chunk nf for q/t block qb: (x[qb] @ W[:, cols])^T
        ps = ps_mm.tile([128, QB], F32, tag="mm", name="ps_qkv")
        for c in range(8):
            nc.tensor.matmul(
                ps,
                lhsT=W_t[:, c, nf * 128:(nf + 1) * 128],
                rhs=xq_t[:, c, qb, :],
                start=(c == 0),
                stop=(c == 7),
            )
        qsl = slice(qb * QB, (qb + 1) * QB)
        if nf < 2:
            nc.vector.tensor_scalar_add(qT[:, nf, qsl], ps, b_t[:, nf:nf + 1])
        else:
            nc.vector.tensor_copy(kT[:, nf - 2, qsl], ps)

    def vn_mm(ps, j, c):
        qb, r = j // 4, j % 4
        nc.tensor.matmul(
            ps,
            lhsT=xq_t[:, c, qb, 128 * r:128 * r + 128],
            rhs=W_t[:, c, 512:768],
            start=(c == 0),
            stop=(c == 7),
        )

    def vn_evac(ps, j):
        for p in range(2):
            # psum cols [128p:128p+64 | +64:+128] -> vnat[j, 0, 0:64] and
            # [j, 1, 64:128] in one strided copy (also f32 -> bf16)
            v0 = vnat[:, p, j, 0, 0:64]
            dst = bass.AP(tensor=v0.tensor, offset=v0.offset,
                          ap=[v0.ap[0], [192, 2], [1, 64]])
            s0 = ps[:, 128 * p:128 * p + 64]
            src = bass.AP(tensor=s0.tensor, offset=s0.offset,
                          ap=[s0.ap[0], [64, 2], [1, 64]])
            nc.vector.tensor_copy(dst, src)

    def vn_chunk(j):
        # V chunk j (128 k-positions) in natural [kpos, vfeat] layout:
        # stationary = x^T slice, moving = W_v columns.  Covers both pairs.
        ps = ps_mm.tile([128, QB], F32, tag="mm", name="ps_vn")
        for c in range(8):
            vn_mm(ps[:, 0:256], j, c)
        vn_evac(ps, j)

    mb = bass.AP(tensor=mask_t.tensor, offset=mask_t.offset,
                 ap=[mask_t.ap[0], [0, 2], [1, 128]])

    def sc(p, j, qb):
        # scores^T + exp for pair p, key chunk j, q block qb (both heads).
        # Returns (pt, q0): pt[:, hl, q0:512] holds exp'd scores.
        r = j - 4 * qb
        q0 = 128 * r if r >= 0 else 0
        ps = ps_s.tile([128, 2, QB], F32, tag="s", name="ps_s_t")
        for hl in range(2):
            nc.tensor.matmul(
                ps[:, hl, q0:QB],
                lhsT=kT[64 * hl:64 * hl + 64, p, j * 128:(j + 1) * 128],
                rhs=qT[64 * hl:64 * hl + 64, p, qb * QB + q0:(qb + 1) * QB],
                start=True,
                stop=True,
            )
        pt = pt_pool.tile([128, 2, QB], BF16, tag="pt", name=f"pt_{p}_{j}_{qb}")
        nc.scalar.activation(pt[:, :, q0:QB], ps[:, :, q0:QB], AF.Exp,
                             scale=0.125)
        if r >= 0:
            # zero the q < k triangle of the diagonal 128x128 (both heads via
            # a broadcast AP over the head dim)
            nc.gpsimd.tensor_mul(pt[:, :, q0:q0 + 128], pt[:, :, q0:q0 + 128],
                                 mb)
        return pt, q0

    def pv_mm(pv_ps, p, hl, j, pt, q0, first, last):
        nc.tensor.matmul(
            pv_ps[:, q0:QB],
            lhsT=vnat[:, p, j, hl, :],
            rhs=pt[:, hl, q0:QB],
            start=first,
            stop=last,
        )

    def norm(pv_ps, p, hl, qb):
        # yT = y / den via reciprocal_approx_fast.  The recip runs over all
        # 128 partitions (cost scales with free size only); the 64 rows that
        # hold y instead of den produce garbage that is never read.
        ysl = slice(64 * hl, 64 * hl + 64)
        dsl = slice(64 - 64 * hl, 128 - 64 * hl)
        rt = rt_pool.tile([128, QB], F32, tag="rt", name="rt")
        nc.vector.reciprocal_approx_fast(out=rt, in_=pv_ps)
        nc.vector.tensor_mul(
            yT[ysl, p, qb * QB:(qb + 1) * QB], pv_ps[ysl, :], rt[dsl, :]
        )

    outT_v = outT.rearrange("(n p) t -> p n t", p=128)

    ob_tiles = {}

    def prj(qb, i):
        # final projection rows [128i:128(i+1)] for q block qb
        qsl = slice(qb * QB, (qb + 1) * QB)
        ps = ps_mm.tile([128, QB], F32, tag="mm", name="ps_o")
        for kc in range(2):
            nc.tensor.matmul(
                ps,
                lhsT=Wp_t[:, kc, i * 128:(i + 1) * 128],
                rhs=yT[:, kc, qsl],
                start=(kc == 0),
                stop=(kc == 1),
            )
        ob = osb_pool.tile([128, QB], BF16, tag="ob", name="ob")
        # alternate the evacuation between DVE and the mostly-idle ScalarE
        if i % 2 == 0:
            nc.vector.tensor_copy(ob, ps)
        else:
            nc.scalar.copy(ob, ps)
        nc.sync.dma_start(out=outT_v[:, i, qsl], in_=ob)

    # ---------------- emission schedule ----------------
    # Units run in order (p=0,qb=0), (1,0), (0,1), (1,1), ... Each unit is a
    # j-sweep of scores+exp with PV lagged LAG slots behind; filler work
    # (next step's qkv/V prep, previous step's projection) is sprinkled into
    # the sweep to keep the PE dense while ACT works through the exps.
    def unit(p, qb, fillers, carry=None):
        # Returns a `finish` closure emitting this unit's PV tail + norms;
        # the caller passes it into the NEXT unit (as `carry`), which emits
        # it right after its first score, so the next unit's scores reach
        # the ACT queue without waiting behind this unit's PV tail.
        js = list(range(4 * qb + 4))
        nslots = len(js)
        lag = 1 if nslots <= 4 else LAG
        nf = len(fillers)
        pts = {}
        pv0 = ps_pv.tile([128, QB], F32, tag="pv", name=f"pv_{p}_{qb}_0")
        pv1 = ps_pv.tile([128, QB], F32, tag="pv", name=f"pv_{p}_{qb}_1")
        pvt = (pv0, pv1)
        done = 0
        for j in js:
            pts[j] = sc(p, j, qb)
            if j == 0 and carry is not None:
                carry()
            want = nf * (j + 1) // nslots
            while done < want:
                fillers[done]()
                done += 1
            if j - lag >= 0:
                jp = j - lag
                pt, q0 = pts.pop(jp)
                for hl in range(2):
                    pv_mm(pvt[hl], p, hl, jp, pt, q0, jp == 0,
                          jp == js[-1])

        def finish():
            # tail: interleave each hl's last PV with the other's norm so
            # the DVE normalize overlaps the remaining PE work
            for jp in range(max(0, len(js) - lag), len(js)):
                pt, q0 = pts.pop(jp)
                last = jp == js[-1]
                pv_mm(pvt[0], p, 0, jp, pt, q0, jp == 0, last)
                if last:
                    norm(pvt[0], p, 0, qb)
                pv_mm(pvt[1], p, 1, jp, pt, q0, jp == 0, last)
                if last:
                    norm(pvt[1], p, 1, qb)
        return finish

    with nc.named_scope("lead"):
        # All x0-dependent matmul work, interleaved per x chunk as the DMAs
        # land: Q/K for both pairs of q block 0 plus V chunks 0..3 -- eight
        # concurrent psum groups (2 ps_mm + 2 ps_pv + 2x2 ps_s bank halves)
        # so the PE is saturated from the first chunk's arrival.
        ps_k0 = ps_mm.tile([128, QB], F32, tag="mm", name="ps_qkv")
        ps_k1 = ps_mm.tile([128, QB], F32, tag="mm", name="ps_qkv")
        # HAM pre-warm: ~3.4us of junk matmul activity on zeroed scratch
        # during the input-DMA wait flips the PE clock to 2.4GHz before the
        # first real lead matmul lands (cold lead chunks are PE-bound).
        for _ in range(8):
            nc.tensor.matmul(ps_k0, lhsT=jw, rhs=jx, start=True, stop=True)
            nc.tensor.matmul(ps_k1, lhsT=jw, rhs=jx, start=True, stop=True)
        ps_q0 = ps_pv.tile([128, QB], F32, tag="pv", name="ps_q0")
        ps_q1 = ps_pv.tile([128, QB], F32, tag="pv", name="ps_q1")
        ps_v01 = ps_s.tile([128, 2, QB], F32, tag="s", name="ps_v01")
        ps_v23 = ps_s.tile([128, 2, QB], F32, tag="s", name="ps_v23")
        vps = [ps_v01[:, 0, 0:256], ps_v01[:, 1, 0:256],
               ps_v23[:, 0, 0:256], ps_v23[:, 1, 0:256]]
        for c in range(8):
            for ps, nf in ((ps_k0, 2), (ps_k1, 3), (ps_q0, 0), (ps_q1, 1)):
                nc.tensor.matmul(
                    ps,
                    lhsT=W_t[:, c, nf * 128:(nf + 1) * 128],
                    rhs=xq_t[:, c, 0, :],
                    start=(c == 0),
                    stop=(c == 7),
                )
            for j in range(4):
                vn_mm(vps[j], j, c)
        nc.scalar.copy(kT[:, 0, 0:QB], ps_k0)
        nc.scalar.copy(kT[:, 1, 0:QB], ps_k1)
        nc.scalar.add(qT[:, 0, 0:QB], ps_q0, b_t[:, 0:1])
        nc.scalar.add(qT[:, 1, 0:QB], ps_q1, b_t[:, 1:2])
        vn_evac(ps_v01[:, 0, :], 0)
        vn_evac(ps_v01[:, 1, :], 1)
        vn_evac(ps_v23[:, 0, :], 2)
        vn_evac(ps_v23[:, 1, :], 3)
        # q block 1's Q/K (x1-gated) ahead of the first unit
        qkv_group(0, 1)
        qkv_group(2, 1)

    def F(fn, *a):
        return lambda: fn(*a)

    # Units in q-block order 1, 2, 3, 0: ascending keeps the x DMA stream
    # ahead of compute, and saving block 0 for last makes the terminal
    # (ACT-bound) unit the smallest one.  Projections fill the ACT-heavy
    # late units.
    plan = [
        (0, 1, [F(vn_chunk, 4), F(vn_chunk, 5), F(qkv_group, 1, 1),
                F(vn_chunk, 6), F(vn_chunk, 7), F(qkv_group, 3, 1)]),
        (1, 1, [F(qkv_group, 0, 2), F(qkv_group, 2, 2)]
            + [F(vn_chunk, j) for j in range(8, 12)]),
        (0, 2, [F(qkv_group, 1, 2), F(qkv_group, 3, 2)]),
        (1, 2, [F(qkv_group, 0, 3), F(qkv_group, 2, 3)]),
        (0, 3, [F(qkv_group, 1, 3), F(qkv_group, 3, 3), F(vn_chunk, 12),
                F(vn_chunk, 13), F(vn_chunk, 14), F(vn_chunk, 15)]
            + [F(prj, 1, i) for i in range(6)]),
        (1, 3, [F(prj, 1, 6), F(prj, 1, 7)]
            + [F(prj, 2, i) for i in range(8)]),
        (0, 0, [F(prj, 3, i) for i in range(4)]),
        (1, 0, [F(prj, 3, i) for i in range(4, 8)]),
    ]
    carry = None
    for p, qb, fillers in plan:
        with nc.named_scope(f"u{p}_{qb}"):
            carry = unit(p, qb, fillers, carry)
    with nc.named_scope("tail"):
        carry()
        junk = ps_pv.tile([128, QB], F32, tag="pv", name="junk")
        for i in range(8):
            if i % 2 == 0:
                nc.tensor.matmul(junk, lhsT=W_t[:, 0, 0:128],
                                 rhs=xq_t[:, 0, 0, :], start=True,
                                 stop=True)
            prj(0, i)
    if dbg is not None:
        nc.sync.dma_start(out=dbg["dbg_q"], in_=qT)
        nc.sync.dma_start(out=dbg["dbg_k"], in_=kT)
        nc.sync.dma_start(out=dbg["dbg_y"], in_=yT)
        nc.sync.dma_start(out=dbg["dbg_v"],
                          in_=vnat.rearrange("p a b c d -> p (a b c d)"))
    ctx.close()


# ---------------------------------------------------------------------------
# host-side wrapper
# ---------------------------------------------------------------------------

_NC_CACHE = {}


def _get_nc():
    if "nc" not in _NC_CACHE:
        _NC_CACHE["nc"] = build_kernel()
    return _NC_CACHE["nc"]


def make_in_maps(x, W_attn, b_attn, W_proj, b_proj):
    # multiplicative causal mask for the diagonal chunk, [k, q]: 1 where q >= k
    mask_np = np.triu(np.ones((128, 128), np.float32)).astype(bf16)
    in_maps = []
    for core in range(N_CORES):
        b = core // 4
        g = core % 4
        cols = np.r_[256 * g:256 * g + 256,
                     1024 + 256 * g:1024 + 256 * g + 256,
                     2048 + 256 * g:2048 + 256 * g + 256]
        in_maps.append({
            "xT": np.ascontiguousarray(x[b].T).astype(bf16),
            "W": np.ascontiguousarray(W_attn[:, cols]).astype(bf16),
            "bq": np.ascontiguousarray(
                b_attn[256 * g:256 * g + 256].reshape(2, 128).T
            ).astype(np.float32),
            "Wp": np.ascontiguousarray(
                W_proj[256 * g:256 * g + 256, :]).astype(bf16),
            "mask": mask_np,
        })
    return in_maps


def kernel(x, W_attn, b_attn, W_proj, b_proj, _trace=False, _trace_kwargs=None):
    x = np.asarray(x, np.float32)
    W_attn = np.asarray(W_attn, np.float32)
    b_attn = np.asarray(b_attn, np.float32)
    W_proj = np.asarray(W_proj, np.float32)
    b_proj = np.asarray(b_proj, np.float32)

    nc = _get_nc()
    in_maps = make_in_maps(x, W_attn, b_attn, W_proj, b_proj)
    res = run_bass_kernel_spmd(
        nc, in_maps, core_ids=list(range(N_CORES)), trace=_trace,
        **(_trace_kwargs or {}),
    )
    B = x.shape[0]
    out = np.zeros((B, T, C), np.float32)
    for core in range(N_CORES):
        b = core // 4
        out[b] += res.results[core]["outT"].T.astype(np.float32)
    # K bias is softmax-invariant (dropped); V bias folds into the epilogue.
    out += (b_proj + b_attn[2 * C:] @ W_proj)[None, None, :]
    if _trace:
        kernel._last_results = res
    return out


if __name__ == "__main__":
    # smoke test: build only
    nc = build_kernel()
    print("built ok")
